# revision 7
# baseline (speedup 1.0000x reference)
"""SNN (soft-nearest-neighbor) contrastive loss on 8 Trainium2 NeuronCores.

Math
----
z = concat(x, y) in R^{8192x128};  d_ij = ||z_i - z_j||.
The row max subtracted in the reference cancels mathematically, so
    S0_i  = sum_{j != i} exp(-d_ij)          (device + host gather)
    dp_i  = d_{i, pair(i)}                   (host, O(N*D))
    loss  = mean_i( -log( exp(-dp_i)/S0_i + tiny ) )   (host, trivial)

Symmetry halving
----------------
d_ij is symmetric; each 128-row block R computes exp tiles for column
blocks R..R+32 only (self + 32 forward, cyclically).  Strip = 4224 cols.
Row sums (fused into the ACT exp via accum_out) cover the WHOLE strip,
including the antipodal block (offset 32), which both partners compute
for their own rows.  Column sums (PE one-hot matmul) cover offsets 1..31
and are scattered on the host into the mirrored rows.  Every unordered
pair is counted exactly once per side.

Device pipeline (one SPMD program, 8 cores, rows sharded 1024/core)
------------------------------------------------------------------
PE assembles Q = u.u^T - hsq_j (bf16 matmul + K=1 rank-1 with -hsq_j
+ identity matmul adding -LARGE on the self diagonal) in PSUM.
ACT reads PSUM directly: w = Sqrt(-Q + hsq_i) via the per-partition
bias AP (one instruction per 1024-col PSUM tile), bf16 out.
Phase 2 (one ACT table switch): E = Exp(-w) over the whole 4224-col
strip in one instruction with accum_out row sums; PE accumulates
column sums of E into a single [10,512] PSUM tile via one-hot lhsT.
DVE only drains the colsum accumulator at the end.
Each core gets column-ROTATED operands so every tile index is a
compile-time constant: one identical program for all 8 cores.
"""

import os
import sys
from contextlib import ExitStack

import numpy as np

_TRN_REPO = os.environ.get("TRN_RL_REPO", "/opt/trn_rl_repo")
if _TRN_REPO not in sys.path:
    sys.path.insert(0, _TRN_REPO)

import ml_dtypes

BF16 = ml_dtypes.bfloat16

B = 4096
D = 128
N = 2 * B            # 8192 rows of z
NCORES = 8
RPC = N // NCORES    # 1024 rows per core
S = RPC // 128       # 8 row-subtiles per core
CT = 512             # matmul moving tile (one PSUM bank = 512 f32)
SL = 4224            # strip length: self block + 32 forward blocks
CW = 4096            # colsum window end (blocks 1..31): [base+128, base+CW)
PT = 1024            # PSUM strip tile columns (2 banks)
UCOLS = 5120         # rotated cols touched: [0, 128*(S-1) + SL) = 5120
NCH = 10             # colsum chunks of 512 covering rotated cols [0, 5120)
LARGE = 16384.0      # diagonal nuke: d2 -> 16384, d -> 128, exp(-128) -> 0

PROFILE = False
LAST_RESULT = None

_cache = {}


def _build_program():
    import concourse.tile as tile
    from concourse import bacc, mybir

    f32 = mybir.dt.float32
    bf16 = mybir.dt.bfloat16
    AF = mybir.ActivationFunctionType

    nc = bacc.Bacc()

    h_ubtr = nc.declare_dram_parameter("ubtr", [128, UCOLS], bf16, isOutput=False)
    h_nhsqj = nc.declare_dram_parameter("nhsqj", [1, UCOLS], bf16, isOutput=False)
    h_dfix = nc.declare_dram_parameter("dfix", [128, CT], bf16, isOutput=False)
    h_ident = nc.declare_dram_parameter("ident", [128, 128], bf16, isOutput=False)
    h_oneh = nc.declare_dram_parameter(
        "oneh", [128, NCH * NCH], bf16, isOutput=False
    )
    h_ones1 = nc.declare_dram_parameter("ones1", [1, 128], bf16, isOutput=False)
    h_hsqp = nc.declare_dram_parameter("hsqp", [128, S], f32, isOutput=False)
    h_s0 = nc.declare_dram_parameter("s0", [128, S], f32, isOutput=True)
    h_cs = nc.declare_dram_parameter("cs", [NCH, CT], f32, isOutput=True)

    # strip for subtile s covers rotated cols [s*128, s*128 + SL)
    with tile.TileContext(nc) as tc, ExitStack() as ctx:
        const = ctx.enter_context(tc.tile_pool(name="const", bufs=1))
        wpool = ctx.enter_context(tc.tile_pool(name="wbuf", bufs=S))
        dpool = ctx.enter_context(tc.tile_pool(name="dump", bufs=2))
        pspool = ctx.enter_context(tc.tile_pool(name="ps", bufs=3, space="PSUM"))
        pstail = ctx.enter_context(tc.tile_pool(name="pst", bufs=1, space="PSUM"))
        cspool = ctx.enter_context(tc.tile_pool(name="cps", bufs=1, space="PSUM"))
        misc = ctx.enter_context(tc.tile_pool(name="misc", bufs=1))

        # small operands first (cheap, needed early)
        t_dfix = const.tile([128, CT], bf16)
        nc.sync.dma_start(out=t_dfix[:], in_=h_dfix[:])
        t_ident = const.tile([128, 128], bf16)
        nc.sync.dma_start(out=t_ident[:], in_=h_ident[:])
        t_oneh = const.tile([128, NCH * NCH], bf16)
        nc.sync.dma_start(out=t_oneh[:], in_=h_oneh[:])
        t_ones1 = const.tile([1, 128], bf16)
        nc.sync.dma_start(out=t_ones1[:], in_=h_ones1[:])
        t_hsqp = const.tile([128, S], f32)
        nc.sync.dma_start(out=t_hsqp[:], in_=h_hsqp[:])
        t_nhsqj = const.tile([1, UCOLS], bf16)
        nc.sync.dma_start(out=t_nhsqj[:], in_=h_nhsqj[:])

        # big operand: fine-grained chunks spread across DMA queues,
        # finest for the first strip's columns, issued first.
        t_ubtr = const.tile([128, UCOLS], bf16)
        edges = [0, 256, 512, 768, 1024, 1536, 2048, 2560, 3072, 3584,
                 4096, 4608, 5120]
        for a, b in zip(edges[:-1], edges[1:]):
            nc.sync.dma_start(out=t_ubtr[:, a:b], in_=h_ubtr[:, a:b])

        t_zero10 = const.tile([128, NCH], bf16)  # zero lhsT for cs warmup
        nc.vector.memset(t_zero10[:], 0.0)
        t_z512 = const.tile([128, CT], bf16)
        nc.vector.memset(t_z512[:], 0.0)

        s0_t = const.tile([128, S], f32)

        # single resident colsum accumulator [NCH, 512]
        cs_acc = cspool.tile([NCH, CT], f32, tag="cs", name="cs_acc")

        # zero the colsum accumulator (matmul with zero weights) and keep
        # the PE busy ~3.5us so the HAM clock gate opens (2.4 GHz) before
        # the first real matmuls arrive
        for rep in range(8):
            nc.tensor.matmul(
                cs_acc[:], t_zero10[:], t_z512[:],
                start=(rep == 0), stop=False, skip_group_check=True,
            )

        # ---- Sqrt phase: PE assembles -Q in PSUM, ACT sqrts it ----
        ws = []
        for s in range(S):
            base = s * 128  # strip start in rotated cols
            w = wpool.tile([128, SL], bf16, tag="w")
            ws.append(w)
            for t in range(4):  # four 1024-col PSUM tiles
                c0 = t * PT
                ps = pspool.tile([128, PT], f32, tag="ps")
                for q0 in range(c0, c0 + PT, CT):
                    nc.tensor.matmul(
                        ps[:, q0 - c0:q0 - c0 + CT],
                        t_ubtr[:, base:base + 128],
                        t_ubtr[:, base + q0:base + q0 + CT],
                        start=True,
                        stop=False,
                    )
                    # rank-1: Q += 1 x (-hsq_j)
                    nc.tensor.matmul(
                        ps[:, q0 - c0:q0 - c0 + CT],
                        t_ones1[:],
                        t_nhsqj[:, base + q0:base + q0 + CT],
                        start=False,
                        stop=not (t == 0 and q0 == 0),
                    )
                    if t == 0 and q0 == 0:
                        # self block: nuke the diagonal (cols [0,128))
                        nc.tensor.matmul(
                            ps[:, 0:CT],
                            t_ident[:],
                            t_dfix[:],
                            start=False,
                            stop=True,
                        )
                # w = sqrt(hsq_i - Q) = d_ij   (bias AP = per-partition hsq_i)
                nc.scalar.activation(
                    out=w[:, c0:c0 + PT],
                    in_=ps[:],
                    func=AF.Sqrt,
                    scale=-1.0,
                    bias=t_hsqp[:, s:s + 1],
                )
            # antipodal 128-col tail
            pst = pstail.tile([128, 128], f32, tag="pst")
            nc.tensor.matmul(
                pst[:],
                t_ubtr[:, base:base + 128],
                t_ubtr[:, base + SL - 128:base + SL],
                start=True,
                stop=False,
            )
            nc.tensor.matmul(
                pst[:],
                t_ones1[:],
                t_nhsqj[:, base + SL - 128:base + SL],
                start=False,
                stop=True,
            )
            nc.scalar.activation(
                out=w[:, SL - 128:SL],
                in_=pst[:],
                func=AF.Sqrt,
                scale=-1.0,
                bias=t_hsqp[:, s:s + 1],
            )

        # ---- Exp phase (one ACT table switch) + column sums ----
        for s in range(S):
            base = s * 128
            w = ws[s]
            dump = dpool.tile([128, SL], bf16, tag="dump")
            # whole strip in one instruction, row sums fused
            nc.scalar.activation(
                out=dump[:],
                in_=w[:],
                func=AF.Exp,
                scale=-1.0,
                accum_out=s0_t[:, s:s + 1],
            )
            # column sums over rotated cols [base+128, base+CW), split at
            # absolute 512 boundaries; chunk j accumulates into cs_acc
            # partition j via a one-hot-column lhsT
            lo = base + 128
            hi = base + CW
            j = lo // CT
            while j * CT < hi:
                a = max(lo, j * CT)
                b = min(hi, (j + 1) * CT)
                nc.tensor.matmul(
                    cs_acc[:, a - j * CT:b - j * CT],
                    t_oneh[:, NCH * j:NCH * (j + 1)],
                    dump[:, a - base:b - base],
                    start=False,
                    stop=False,
                    skip_group_check=True,
                )
                j += 1

        # drain colsum accumulator: PSUM -> SBUF -> DRAM
        sb = misc.tile([NCH, CT], f32, tag="csdrain")
        nc.vector.tensor_copy(sb[:], cs_acc[:])
        nc.sync.dma_start(out=h_cs[:], in_=sb[:])
        nc.sync.dma_start(out=h_s0[:], in_=s0_t[:])

    nc.finalize()
    return nc


def get_program():
    if "nc" not in _cache:
        _cache["nc"] = _build_program()
    return _cache["nc"]


def make_in_maps(x, y):
    """Host-side prep: build the per-core (column-rotated) operand arrays."""
    x = np.asarray(x, dtype=np.float32)
    y = np.asarray(y, dtype=np.float32)
    z = np.concatenate([x, y], axis=0)  # [N, D]

    u = (np.float32(np.sqrt(2.0)) * z).astype(BF16)
    uf = u.astype(np.float32)
    hsq = np.float32(0.5) * (uf * uf).sum(axis=1, dtype=np.float32)  # ||u||^2/2

    ubt = np.ascontiguousarray(u.T)  # [D, N] bf16
    nhsq_bf = (-hsq).astype(BF16)

    dfix = np.zeros((128, CT), dtype=BF16)
    idx = np.arange(128)
    dfix[idx, idx] = BF16(-LARGE)
    ident = np.eye(128, dtype=BF16)
    # block j of oneh is the [128, NCH] lhsT whose column j is all ones:
    # out[j, c] += sum_p E[p, c]; other cs rows accumulate zeros
    oneh = np.zeros((128, NCH * NCH), dtype=BF16)
    for j in range(NCH):
        oneh[:, NCH * j + j] = BF16(1.0)
    ones1 = np.ones((1, 128), dtype=BF16)

    in_maps = []
    for c in range(NCORES):
        r0 = c * RPC
        rows = np.arange(r0, r0 + RPC)

        def rot(a):
            return np.ascontiguousarray(np.roll(a, -r0, axis=-1)[..., :UCOLS])

        def pcol(vec, sel):  # [RPC] values -> [128, S] per-partition layout
            return np.ascontiguousarray(vec[sel].reshape(S, 128).T)

        in_maps.append(
            {
                "ubtr": rot(ubt),
                "nhsqj": rot(nhsq_bf[None, :]),
                "dfix": dfix,
                "ident": ident,
                "oneh": oneh,
                "ones1": ones1,
                "hsqp": pcol(hsq, rows),
            }
        )
    return in_maps


def finish_on_host(results, x, y):
    """Gather per-core row sums + column sums; final loss with host dp."""
    S0 = np.zeros(N, dtype=np.float64)
    for c in range(NCORES):
        r0 = c * RPC
        s0 = np.asarray(results[c]["s0"], dtype=np.float64)  # [128, S]
        cs = np.asarray(results[c]["cs"], dtype=np.float64)  # [NCH, CT]
        S0[r0:r0 + RPC] += s0.T.reshape(-1)
        # accumulated column sums: rotated col r in [128, 4992) holds the
        # core's total colsum for global row (r0 + r) mod N
        csf = cs.reshape(-1)
        rot = np.arange(128, (S - 1) * 128 + CW)
        gidx = (r0 + rot) % N
        S0[gidx] += csf[rot]

    z = np.concatenate([np.asarray(x, np.float64), np.asarray(y, np.float64)])
    dp = np.sqrt(((z[:B] - z[B:]) ** 2).sum(axis=1))
    DP = np.concatenate([dp, dp])

    tiny = float(np.finfo(np.float32).tiny)
    num = np.exp(-DP)
    loss = -np.log(num / S0 + tiny)
    return np.asarray(loss.mean(), dtype=np.float32)


def kernel(x, y):
    global LAST_RESULT
    from concourse.bass_utils import run_bass_kernel_spmd

    nc = get_program()
    in_maps = make_in_maps(x, y)
    res = run_bass_kernel_spmd(
        nc, in_maps, list(range(NCORES)), trace=PROFILE
    )
    LAST_RESULT = res
    return finish_on_host(res.results, x, y)


# revision 9
# speedup vs baseline: 1.2126x; 1.2126x over previous
"""SNN (soft-nearest-neighbor) contrastive loss on 8 Trainium2 NeuronCores.

Math
----
z = concat(x, y) in R^{8192x128};  d_ij = ||z_i - z_j||.
The row max subtracted in the reference cancels mathematically, so
    S0_i  = sum_{j != i} exp(-d_ij)          (device + host gather)
    dp_i  = d_{i, pair(i)}                   (host, O(N*D))
    loss  = mean_i( -log( exp(-dp_i)/S0_i + tiny ) )   (host, trivial)

Symmetry halving
----------------
d_ij is symmetric; each 128-row block R computes exp tiles for column
blocks R..R+32 only (self + 32 forward, cyclically).  Strip = 4224 cols.
Row sums (fused into the ACT exp via accum_out) cover the WHOLE strip,
including the antipodal block (offset 32), which both partners compute
for their own rows.  Column sums (PE one-hot matmul into a single
[10,512] PSUM accumulator) cover offsets 1..31 and are scattered on the
host into the mirrored rows.  Every unordered pair counts exactly once.

Device pipeline (one SPMD program, 8 cores, rows sharded 1024/core)
------------------------------------------------------------------
PE: bf16 matmul u^T u into PSUM (+ FD-128 identity matmul adding -16384
on the self diagonal).  DVE: v = (hsq_i - PSUM) + hsq_j ... i.e.
scalar_tensor_tensor assembles -d2 strips into SBUF f32.  ACT: one
Sqrt per 4224-col strip (bf16 out), then one Exp per strip with fused
accum_out row sums (a single sqrt->exp table switch for the whole
kernel).  Each core gets column-ROTATED operands so every tile index
is a compile-time constant: one identical program for all 8 cores.
"""

import os
import sys
from contextlib import ExitStack

import numpy as np

_TRN_REPO = os.environ.get("TRN_RL_REPO", "/opt/trn_rl_repo")
if _TRN_REPO not in sys.path:
    sys.path.insert(0, _TRN_REPO)

import ml_dtypes

BF16 = ml_dtypes.bfloat16

B = 4096
D = 128
N = 2 * B            # 8192 rows of z
NCORES = 8
RPC = N // NCORES    # 1024 rows per core
S = RPC // 128       # 8 row-subtiles per core
CT = 512             # matmul moving tile (one PSUM bank = 512 f32)
SL = 4224            # strip length: self block + 32 forward blocks
CW = 4096            # colsum window end (blocks 1..31): [base+128, base+CW)
PT = 1024            # PSUM strip tile columns (2 banks)
UCOLS = 5120         # rotated cols touched: [0, 128*(S-1) + SL) = 5120
NCH = 10             # colsum chunks of 512 covering rotated cols [0, 5120)
LARGE = 16384.0      # diagonal nuke: d2 -> 16384, d -> 128, exp(-128) -> 0

PROFILE = False
LAST_RESULT = None

_cache = {}


def _build_program():
    import concourse.tile as tile
    from concourse import bacc, mybir

    f32 = mybir.dt.float32
    f16 = mybir.dt.float16
    bf16 = mybir.dt.bfloat16
    AF = mybir.ActivationFunctionType
    OP = mybir.AluOpType

    nc = bacc.Bacc()

    h_ubtr = nc.declare_dram_parameter("ubtr", [128, UCOLS], bf16, isOutput=False)
    h_hsqjb = nc.declare_dram_parameter("hsqjb", [128, UCOLS], f16, isOutput=False)
    h_dfix = nc.declare_dram_parameter("dfix", [128, 128], bf16, isOutput=False)
    h_ident = nc.declare_dram_parameter("ident", [128, 128], bf16, isOutput=False)
    h_oneh = nc.declare_dram_parameter(
        "oneh", [128, NCH * NCH], bf16, isOutput=False
    )
    h_hsqp = nc.declare_dram_parameter("hsqp", [128, S], f32, isOutput=False)
    h_s0 = nc.declare_dram_parameter("s0", [128, S], f32, isOutput=True)
    h_cs = nc.declare_dram_parameter("cs", [NCH, CT], f32, isOutput=True)

    # strip for subtile s covers rotated cols [s*128, s*128 + SL)
    with tile.TileContext(nc) as tc, ExitStack() as ctx:
        const = ctx.enter_context(tc.tile_pool(name="const", bufs=1))
        vpool = ctx.enter_context(tc.tile_pool(name="vbuf", bufs=2))
        wpool = ctx.enter_context(tc.tile_pool(name="wbuf", bufs=S))
        dpool = ctx.enter_context(tc.tile_pool(name="dump", bufs=2))
        pspool = ctx.enter_context(tc.tile_pool(name="ps", bufs=3, space="PSUM"))
        pstail = ctx.enter_context(tc.tile_pool(name="pst", bufs=1, space="PSUM"))
        cspool = ctx.enter_context(tc.tile_pool(name="cps", bufs=1, space="PSUM"))
        misc = ctx.enter_context(tc.tile_pool(name="misc", bufs=1))

        # small operands first (cheap, needed early)
        t_dfix = const.tile([128, 128], bf16)
        nc.sync.dma_start(out=t_dfix[:], in_=h_dfix[:])
        t_ident = const.tile([128, 128], bf16)
        nc.sync.dma_start(out=t_ident[:], in_=h_ident[:])
        t_oneh = const.tile([128, NCH * NCH], bf16)
        nc.sync.dma_start(out=t_oneh[:], in_=h_oneh[:])
        t_hsqp = const.tile([128, S], f32)
        nc.sync.dma_start(out=t_hsqp[:], in_=h_hsqp[:])

        # big operands: fine-grained chunks spread across DMA queues,
        # finest for the first strip's columns, issued first.
        t_ubtr = const.tile([128, UCOLS], bf16)
        t_hsqjb = const.tile([128, UCOLS], f16)
        edges = [0, 256, 512, 768, 1024, 1536, 2048, 2560, 3072, 3584,
                 4096, 4608, 5120]
        for a, b in zip(edges[:-1], edges[1:]):
            nc.sync.dma_start(out=t_ubtr[:, a:b], in_=h_ubtr[:, a:b])
            nc.sync.dma_start(out=t_hsqjb[:, a:b], in_=h_hsqjb[:, a:b])

        t_zero10 = const.tile([128, NCH], bf16)
        nc.vector.memset(t_zero10[:], 0.0)
        t_z512 = const.tile([128, CT], bf16)
        nc.vector.memset(t_z512[:], 0.0)

        s0_t = const.tile([128, S], f32)

        # single resident colsum accumulator [NCH, 512]
        cs_acc = cspool.tile([NCH, CT], f32, tag="cs", name="cs_acc")

        # zero the colsum accumulator (matmul with zero weights) and keep
        # the PE busy ~3.5us so the HAM clock gate opens (2.4 GHz) before
        # the first real matmuls arrive
        for rep in range(8):
            nc.tensor.matmul(
                cs_acc[:], t_zero10[:], t_z512[:],
                start=(rep == 0), stop=False, skip_group_check=True,
            )

        # ---- Sqrt phase: PE u.u -> PSUM, DVE assembles -d2, ACT sqrts ----
        ws = []
        for s in range(S):
            base = s * 128  # strip start in rotated cols
            v = vpool.tile([128, SL], f32, tag="v")
            w = wpool.tile([128, SL], bf16, tag="w")
            ws.append(w)
            for t in range(4):  # four 1024-col PSUM tiles
                c0 = t * PT
                ps = pspool.tile([128, PT], f32, tag="ps")
                for q0 in range(c0, c0 + PT, CT):
                    nc.tensor.matmul(
                        ps[:, q0 - c0:q0 - c0 + CT],
                        t_ubtr[:, base:base + 128],
                        t_ubtr[:, base + q0:base + q0 + CT],
                        start=True,
                        stop=not (t == 0 and q0 == 0),
                    )
                    if t == 0 and q0 == 0:
                        # self block: nuke the diagonal (cols [0,128))
                        nc.tensor.matmul(
                            ps[:, 0:128],
                            t_ident[:],
                            t_dfix[:],
                            start=False,
                            stop=True,
                            skip_group_check=True,
                        )
                # v = (P - hsq_i) - hsq_j = -d2
                nc.vector.scalar_tensor_tensor(
                    out=v[:, c0:c0 + PT],
                    in0=ps[:],
                    scalar=t_hsqp[:, s:s + 1],
                    in1=t_hsqjb[:, base + c0:base + c0 + PT],
                    op0=OP.subtract,
                    op1=OP.subtract,
                )
            # antipodal 128-col tail
            pst = pstail.tile([128, 128], f32, tag="pst")
            nc.tensor.matmul(
                pst[:],
                t_ubtr[:, base:base + 128],
                t_ubtr[:, base + SL - 128:base + SL],
                start=True,
                stop=True,
            )
            nc.vector.scalar_tensor_tensor(
                out=v[:, SL - 128:SL],
                in0=pst[:],
                scalar=t_hsqp[:, s:s + 1],
                in1=t_hsqjb[:, base + SL - 128:base + SL],
                op0=OP.subtract,
                op1=OP.subtract,
            )
            # w = sqrt(-v) = d_ij, one instruction per strip
            nc.scalar.activation(
                out=w[:],
                in_=v[:],
                func=AF.Sqrt,
                scale=-1.0,
            )

        # ---- Exp phase (one ACT table switch) + column sums ----
        for s in range(S):
            base = s * 128
            w = ws[s]
            dump = dpool.tile([128, SL], bf16, tag="dump")
            # whole strip in one instruction, row sums fused
            nc.scalar.activation(
                out=dump[:],
                in_=w[:],
                func=AF.Exp,
                scale=-1.0,
                accum_out=s0_t[:, s:s + 1],
            )
            # column sums over rotated cols [base+128, base+CW), split at
            # absolute 512 boundaries; chunk j accumulates into cs_acc
            # partition j via a one-hot-column lhsT
            lo = base + 128
            hi = base + CW
            j = lo // CT
            while j * CT < hi:
                a = max(lo, j * CT)
                b = min(hi, (j + 1) * CT)
                nc.tensor.matmul(
                    cs_acc[:, a - j * CT:b - j * CT],
                    t_oneh[:, NCH * j:NCH * (j + 1)],
                    dump[:, a - base:b - base],
                    start=False,
                    stop=False,
                    skip_group_check=True,
                )
                j += 1

        # drain colsum accumulator: PSUM -> SBUF -> DRAM
        sb = misc.tile([NCH, CT], f32, tag="csdrain")
        nc.vector.tensor_copy(sb[:], cs_acc[:])
        nc.sync.dma_start(out=h_cs[:], in_=sb[:])
        nc.sync.dma_start(out=h_s0[:], in_=s0_t[:])

    nc.finalize()
    return nc


def get_program():
    if "nc" not in _cache:
        _cache["nc"] = _build_program()
    return _cache["nc"]


def make_in_maps(x, y):
    """Host-side prep: build the per-core (column-rotated) operand arrays."""
    x = np.asarray(x, dtype=np.float32)
    y = np.asarray(y, dtype=np.float32)
    z = np.concatenate([x, y], axis=0)  # [N, D]

    u = (np.float32(np.sqrt(2.0)) * z).astype(BF16)
    uf = u.astype(np.float32)
    hsq = np.float32(0.5) * (uf * uf).sum(axis=1, dtype=np.float32)  # ||u||^2/2

    ubt = np.ascontiguousarray(u.T)  # [D, N] bf16
    hsq_f16 = hsq.astype(np.float16)

    dfix = np.zeros((128, 128), dtype=BF16)
    idx = np.arange(128)
    dfix[idx, idx] = BF16(-LARGE)
    ident = np.eye(128, dtype=BF16)
    # block j of oneh is the [128, NCH] lhsT whose column j is all ones:
    # out[j, c] += sum_p E[p, c]; other cs rows accumulate zeros
    oneh = np.zeros((128, NCH * NCH), dtype=BF16)
    for j in range(NCH):
        oneh[:, NCH * j + j] = BF16(1.0)

    in_maps = []
    for c in range(NCORES):
        r0 = c * RPC
        rows = np.arange(r0, r0 + RPC)

        def rot(a):
            return np.ascontiguousarray(np.roll(a, -r0, axis=-1)[..., :UCOLS])

        def pcol(vec, sel):  # [RPC] values -> [128, S] per-partition layout
            return np.ascontiguousarray(vec[sel].reshape(S, 128).T)

        in_maps.append(
            {
                "ubtr": rot(ubt),
                "hsqjb": np.ascontiguousarray(
                    np.broadcast_to(
                        np.roll(hsq_f16, -r0)[None, :UCOLS], (128, UCOLS)
                    )
                ),
                "dfix": dfix,
                "ident": ident,
                "oneh": oneh,
                "hsqp": pcol(hsq, rows),
            }
        )
    return in_maps


def finish_on_host(results, x, y):
    """Gather per-core row sums + column sums; final loss with host dp."""
    S0 = np.zeros(N, dtype=np.float64)
    for c in range(NCORES):
        r0 = c * RPC
        s0 = np.asarray(results[c]["s0"], dtype=np.float64)  # [128, S]
        cs = np.asarray(results[c]["cs"], dtype=np.float64)  # [NCH, CT]
        S0[r0:r0 + RPC] += s0.T.reshape(-1)
        # accumulated column sums: rotated col r in [128, 4992) holds the
        # core's total colsum for global row (r0 + r) mod N
        csf = cs.reshape(-1)
        rot = np.arange(128, (S - 1) * 128 + CW)
        gidx = (r0 + rot) % N
        S0[gidx] += csf[rot]

    z = np.concatenate([np.asarray(x, np.float64), np.asarray(y, np.float64)])
    dp = np.sqrt(((z[:B] - z[B:]) ** 2).sum(axis=1))
    DP = np.concatenate([dp, dp])

    tiny = float(np.finfo(np.float32).tiny)
    num = np.exp(-DP)
    loss = -np.log(num / S0 + tiny)
    return np.asarray(loss.mean(), dtype=np.float32)


def kernel(x, y):
    global LAST_RESULT
    from concourse.bass_utils import run_bass_kernel_spmd

    nc = get_program()
    in_maps = make_in_maps(x, y)
    res = run_bass_kernel_spmd(
        nc, in_maps, list(range(NCORES)), trace=PROFILE
    )
    LAST_RESULT = res
    return finish_on_host(res.results, x, y)


# revision 13
# speedup vs baseline: 1.2308x; 1.0150x over previous
"""SNN (soft-nearest-neighbor) contrastive loss on 8 Trainium2 NeuronCores.

Math
----
z = concat(x, y) in R^{8192x128};  d_ij = ||z_i - z_j||.
The row max subtracted in the reference cancels mathematically, so
    S0_i  = sum_{j != i} exp(-d_ij)          (device + host gather)
    dp_i  = d_{i, pair(i)}                   (host, O(N*D))
    loss  = mean_i( -log( exp(-dp_i)/S0_i + tiny ) )   (host, trivial)

Symmetry halving
----------------
d_ij is symmetric; each 128-row block R computes exp tiles for column
blocks R..R+32 only (self + 32 forward, cyclically).  Strip = 4224 cols.
Row sums (fused into the ACT exp via accum_out) cover the WHOLE strip,
including the antipodal block (offset 32), which both partners compute
for their own rows.  Column sums (PE one-hot matmul into a single
[10,512] PSUM accumulator) cover offsets 1..31 and are scattered on the
host into the mirrored rows.  Every unordered pair counts exactly once.

Device pipeline (one SPMD program, 8 cores, rows sharded 1024/core)
------------------------------------------------------------------
PE: bf16 matmul u^T u into PSUM (+ FD-128 identity matmul adding -16384
on the self diagonal).  DVE: v = (hsq_i - PSUM) + hsq_j ... i.e.
scalar_tensor_tensor assembles -d2 strips into SBUF f32.  ACT: one
Sqrt per 4224-col strip (bf16 out), then one Exp per strip with fused
accum_out row sums (a single sqrt->exp table switch for the whole
kernel).  Each core gets column-ROTATED operands so every tile index
is a compile-time constant: one identical program for all 8 cores.
"""

import os
import sys
from contextlib import ExitStack

import numpy as np

_TRN_REPO = os.environ.get("TRN_RL_REPO", "/opt/trn_rl_repo")
if _TRN_REPO not in sys.path:
    sys.path.insert(0, _TRN_REPO)

import ml_dtypes

BF16 = ml_dtypes.bfloat16

B = 4096
D = 128
N = 2 * B            # 8192 rows of z
NCORES = 8
RPC = N // NCORES    # 1024 rows per core
S = RPC // 128       # 8 row-subtiles per core
CT = 512             # matmul moving tile (one PSUM bank = 512 f32)
SL = 4224            # strip length: self block + 32 forward blocks
CW = 4096            # colsum window end (blocks 1..31): [base+128, base+CW)
PT = 1024            # PSUM strip tile columns (2 banks)
UCOLS = 5120         # rotated cols touched: [0, 128*(S-1) + SL) = 5120
NCH = 10             # colsum chunks of 512 covering rotated cols [0, 5120)
LARGE = 16384.0      # diagonal nuke: d2 -> 16384, d -> 128, exp(-128) -> 0

PROFILE = False
LAST_RESULT = None

_cache = {}


def _build_program():
    import concourse.tile as tile
    from bass_rust import add_dep_helper
    from concourse import bacc, mybir

    f32 = mybir.dt.float32
    f16 = mybir.dt.float16
    bf16 = mybir.dt.bfloat16
    AF = mybir.ActivationFunctionType
    OP = mybir.AluOpType

    nc = bacc.Bacc()

    h_ubtr = nc.declare_dram_parameter("ubtr", [128, UCOLS], bf16, isOutput=False)
    h_hsqjb = nc.declare_dram_parameter("hsqjb", [128, UCOLS], f16, isOutput=False)
    h_dfix = nc.declare_dram_parameter("dfix", [128, 128], bf16, isOutput=False)
    h_ident = nc.declare_dram_parameter("ident", [128, 128], bf16, isOutput=False)
    h_oneh = nc.declare_dram_parameter(
        "oneh", [128, NCH * NCH], bf16, isOutput=False
    )
    h_hsqp = nc.declare_dram_parameter("hsqp", [128, S], f32, isOutput=False)
    h_s0 = nc.declare_dram_parameter("s0", [128, S], f32, isOutput=True)
    h_cs = nc.declare_dram_parameter("cs", [NCH, CT], f32, isOutput=True)

    # strip for subtile s covers rotated cols [s*128, s*128 + SL)
    with tile.TileContext(nc) as tc, ExitStack() as ctx:
        const = ctx.enter_context(tc.tile_pool(name="const", bufs=1))
        vpool = ctx.enter_context(tc.tile_pool(name="vbuf", bufs=2))
        wpool = ctx.enter_context(tc.tile_pool(name="wbuf", bufs=S))
        dpool = ctx.enter_context(tc.tile_pool(name="dump", bufs=2))
        pspool = ctx.enter_context(tc.tile_pool(name="ps", bufs=3, space="PSUM"))
        pstail = ctx.enter_context(tc.tile_pool(name="pst", bufs=1, space="PSUM"))
        cspool = ctx.enter_context(tc.tile_pool(name="cps", bufs=1, space="PSUM"))
        misc = ctx.enter_context(tc.tile_pool(name="misc", bufs=1))

        # small operands first (cheap, needed early)
        t_dfix = const.tile([128, 128], bf16)
        nc.sync.dma_start(out=t_dfix[:], in_=h_dfix[:])
        t_ident = const.tile([128, 128], bf16)
        nc.sync.dma_start(out=t_ident[:], in_=h_ident[:])
        t_oneh = const.tile([128, NCH * NCH], bf16)
        nc.sync.dma_start(out=t_oneh[:], in_=h_oneh[:])
        t_hsqp = const.tile([128, S], f32)
        nc.sync.dma_start(out=t_hsqp[:], in_=h_hsqp[:])

        # big operands: fine-grained chunks spread across DMA queues,
        # finest for the first strip's columns, issued first.
        t_ubtr = const.tile([128, UCOLS], bf16)
        t_hsqjb = const.tile([128, UCOLS], f16)
        edges = [0, 256, 512, 768, 1024, 1536, 2048, 2560, 3072, 3584,
                 4096, 4608, 5120]
        for a, b in zip(edges[:-1], edges[1:]):
            nc.sync.dma_start(out=t_ubtr[:, a:b], in_=h_ubtr[:, a:b])
            nc.sync.dma_start(out=t_hsqjb[:, a:b], in_=h_hsqjb[:, a:b])

        t_zero10 = const.tile([128, NCH], bf16)
        nc.vector.memset(t_zero10[:], 0.0)
        t_z512 = const.tile([128, CT], bf16)
        nc.vector.memset(t_z512[:], 0.0)

        s0_t = const.tile([128, S], f32)

        # single resident colsum accumulator [NCH, 512]
        cs_acc = cspool.tile([NCH, CT], f32, tag="cs", name="cs_acc")

        # zero the colsum accumulator (matmul with zero weights) and keep
        # the PE busy ~3.5us so the HAM clock gate opens (2.4 GHz) before
        # the first real matmuls arrive
        for rep in range(8):
            nc.tensor.matmul(
                cs_acc[:], t_zero10[:], t_z512[:],
                start=(rep == 0), stop=False, skip_group_check=True,
            )

        # ---- Sqrt phase: PE u.u -> PSUM, DVE assembles -d2, ACT sqrts ----
        ws = []
        last_sqrt = None
        for s in range(S):
            base = s * 128  # strip start in rotated cols
            v = vpool.tile([128, SL], f32, tag="v")
            w = wpool.tile([128, SL], bf16, tag="w")
            ws.append(w)
            for t in range(4):  # four 1024-col PSUM tiles
                c0 = t * PT
                ps = pspool.tile([128, PT], f32, tag="ps")
                for q0 in range(c0, c0 + PT, CT):
                    nc.tensor.matmul(
                        ps[:, q0 - c0:q0 - c0 + CT],
                        t_ubtr[:, base:base + 128],
                        t_ubtr[:, base + q0:base + q0 + CT],
                        start=True,
                        stop=not (t == 0 and q0 == 0),
                    )
                    if t == 0 and q0 == 0:
                        # self block: nuke the diagonal (cols [0,128))
                        nc.tensor.matmul(
                            ps[:, 0:128],
                            t_ident[:],
                            t_dfix[:],
                            start=False,
                            stop=True,
                            skip_group_check=True,
                        )
                # v = (P - hsq_i) - hsq_j = -d2
                nc.vector.scalar_tensor_tensor(
                    out=v[:, c0:c0 + PT],
                    in0=ps[:],
                    scalar=t_hsqp[:, s:s + 1],
                    in1=t_hsqjb[:, base + c0:base + c0 + PT],
                    op0=OP.subtract,
                    op1=OP.subtract,
                )
            # antipodal 128-col tail
            pst = pstail.tile([128, 128], f32, tag="pst")
            nc.tensor.matmul(
                pst[:],
                t_ubtr[:, base:base + 128],
                t_ubtr[:, base + SL - 128:base + SL],
                start=True,
                stop=True,
            )
            nc.vector.scalar_tensor_tensor(
                out=v[:, SL - 128:SL],
                in0=pst[:],
                scalar=t_hsqp[:, s:s + 1],
                in1=t_hsqjb[:, base + SL - 128:base + SL],
                op0=OP.subtract,
                op1=OP.subtract,
            )
            # w = sqrt(-v) = d_ij, one instruction per strip
            last_sqrt = nc.scalar.activation(
                out=w[:],
                in_=v[:],
                func=AF.Sqrt,
                scale=-1.0,
            )

        # ---- Exp phase (one ACT table switch) + column sums ----
        for s in range(S):
            base = s * 128
            w = ws[s]
            dump = dpool.tile([128, SL], bf16, tag="dump")
            # whole strip in one instruction, row sums fused
            e = nc.scalar.activation(
                out=dump[:],
                in_=w[:],
                func=AF.Exp,
                scale=-1.0,
                accum_out=s0_t[:, s:s + 1],
            )
            if last_sqrt is not None:
                add_dep_helper(
                    e.ins, last_sqrt.ins, sync=False,
                    reason="ACT table phase: exp after all sqrts",
                )
            # column sums over rotated cols [base+128, base+CW), split at
            # absolute 512 boundaries; chunk j accumulates into cs_acc
            # partition j via a one-hot-column lhsT
            lo = base + 128
            hi = base + CW
            j = lo // CT
            while j * CT < hi:
                a = max(lo, j * CT)
                b = min(hi, (j + 1) * CT)
                nc.tensor.matmul(
                    cs_acc[:, a - j * CT:b - j * CT],
                    t_oneh[:, NCH * j:NCH * (j + 1)],
                    dump[:, a - base:b - base],
                    start=False,
                    stop=False,
                    skip_group_check=True,
                )
                j += 1

        # drain colsum accumulator: PSUM -> SBUF -> DRAM
        sb = misc.tile([NCH, CT], f32, tag="csdrain")
        nc.vector.tensor_copy(sb[:], cs_acc[:])
        nc.sync.dma_start(out=h_cs[:], in_=sb[:])
        nc.sync.dma_start(out=h_s0[:], in_=s0_t[:])

    nc.finalize()
    return nc


def get_program():
    if "nc" not in _cache:
        _cache["nc"] = _build_program()
    return _cache["nc"]


def make_in_maps(x, y):
    """Host-side prep: build the per-core (column-rotated) operand arrays."""
    x = np.asarray(x, dtype=np.float32)
    y = np.asarray(y, dtype=np.float32)
    z = np.concatenate([x, y], axis=0)  # [N, D]

    u = (np.float32(np.sqrt(2.0)) * z).astype(BF16)
    uf = u.astype(np.float32)
    hsq = np.float32(0.5) * (uf * uf).sum(axis=1, dtype=np.float32)  # ||u||^2/2

    ubt = np.ascontiguousarray(u.T)  # [D, N] bf16
    hsq_f16 = hsq.astype(np.float16)

    dfix = np.zeros((128, 128), dtype=BF16)
    idx = np.arange(128)
    dfix[idx, idx] = BF16(-LARGE)
    ident = np.eye(128, dtype=BF16)
    # block j of oneh is the [128, NCH] lhsT whose column j is all ones:
    # out[j, c] += sum_p E[p, c]; other cs rows accumulate zeros
    oneh = np.zeros((128, NCH * NCH), dtype=BF16)
    for j in range(NCH):
        oneh[:, NCH * j + j] = BF16(1.0)

    in_maps = []
    for c in range(NCORES):
        r0 = c * RPC
        rows = np.arange(r0, r0 + RPC)

        def rot(a):
            return np.ascontiguousarray(np.roll(a, -r0, axis=-1)[..., :UCOLS])

        def pcol(vec, sel):  # [RPC] values -> [128, S] per-partition layout
            return np.ascontiguousarray(vec[sel].reshape(S, 128).T)

        in_maps.append(
            {
                "ubtr": rot(ubt),
                "hsqjb": np.ascontiguousarray(
                    np.broadcast_to(
                        np.roll(hsq_f16, -r0)[None, :UCOLS], (128, UCOLS)
                    )
                ),
                "dfix": dfix,
                "ident": ident,
                "oneh": oneh,
                "hsqp": pcol(hsq, rows),
            }
        )
    return in_maps


def finish_on_host(results, x, y):
    """Gather per-core row sums + column sums; final loss with host dp."""
    S0 = np.zeros(N, dtype=np.float64)
    for c in range(NCORES):
        r0 = c * RPC
        s0 = np.asarray(results[c]["s0"], dtype=np.float64)  # [128, S]
        cs = np.asarray(results[c]["cs"], dtype=np.float64)  # [NCH, CT]
        S0[r0:r0 + RPC] += s0.T.reshape(-1)
        # accumulated column sums: rotated col r in [128, 4992) holds the
        # core's total colsum for global row (r0 + r) mod N
        csf = cs.reshape(-1)
        rot = np.arange(128, (S - 1) * 128 + CW)
        gidx = (r0 + rot) % N
        S0[gidx] += csf[rot]

    z = np.concatenate([np.asarray(x, np.float64), np.asarray(y, np.float64)])
    dp = np.sqrt(((z[:B] - z[B:]) ** 2).sum(axis=1))
    DP = np.concatenate([dp, dp])

    tiny = float(np.finfo(np.float32).tiny)
    num = np.exp(-DP)
    loss = -np.log(num / S0 + tiny)
    return np.asarray(loss.mean(), dtype=np.float32)


def kernel(x, y):
    global LAST_RESULT
    from concourse.bass_utils import run_bass_kernel_spmd

    nc = get_program()
    in_maps = make_in_maps(x, y)
    res = run_bass_kernel_spmd(
        nc, in_maps, list(range(NCORES)), trace=PROFILE
    )
    LAST_RESULT = res
    return finish_on_host(res.results, x, y)


# revision 14
# speedup vs baseline: 1.4942x; 1.2140x over previous
"""SNN (soft-nearest-neighbor) contrastive loss on 8 Trainium2 NeuronCores.

Math
----
z = concat(x, y) in R^{8192x128};  d_ij = ||z_i - z_j||.
The row max subtracted in the reference cancels mathematically, so
    S0_i  = sum_{j != i} exp(-d_ij)          (device + host gather)
    dp_i  = d_{i, pair(i)}                   (host, O(N*D))
    loss  = mean_i( -log( exp(-dp_i)/S0_i + tiny ) )   (host, trivial)

Symmetry halving
----------------
d_ij is symmetric; each 128-row block R computes exp tiles for column
blocks R..R+32 only (self + 32 forward, cyclically).  Strip = 4224 cols.
Row sums cover the WHOLE strip (the antipodal block, offset 32, is
computed by both partners for their own rows).  Column sums (PE one-hot
matmul into a single [10,512] PSUM accumulator) cover offsets 1..31 and
are scattered on the host into the mirrored rows.

Device pipeline (one SPMD program, 8 cores, rows sharded 1024/core)
------------------------------------------------------------------
PE: fp8 DoubleRow matmuls with 65-row k-tiles compute
      Q = u.u^T - hsq_j      (u split into 2x64 dims; the 65th row of
k-tile 0/1 carries ones x -hsq_hi / ones x -hsq_lo, giving the hsq_j
fold at fp16-ish precision for free), plus an FD-128 bf16 identity
matmul adding -LARGE on the self diagonal.
ACT: w = Sqrt(-Q + hsq_i) straight from PSUM via the per-partition
bias AP (bf16 out).  Exp is split: N_ACT strips run on ACT
(exp + fused accum_out row sums); N_DVE strips run on DVE via the
Schraudolph bit trick -- the sqrt for those strips is scaled by A16^2
so codes = int16(B16 - A16*w) come from one scalar_tensor_tensor, and
a second tensor_scalar pass over the bitcast-bf16 codes yields row
sums via accum_out.  PE accumulates column sums of every strip.
Each core gets column-ROTATED operands so every tile index is a
compile-time constant: one identical program for all 8 cores.
"""

import os
import sys
from contextlib import ExitStack

import numpy as np

_TRN_REPO = os.environ.get("TRN_RL_REPO", "/opt/trn_rl_repo")
if _TRN_REPO not in sys.path:
    sys.path.insert(0, _TRN_REPO)

import ml_dtypes

BF16 = ml_dtypes.bfloat16

B = 4096
D = 128
N = 2 * B            # 8192 rows of z
NCORES = 8
RPC = N // NCORES    # 1024 rows per core
S = RPC // 128       # 8 row-subtiles per core
CT = 512             # matmul moving tile (one PSUM bank = 512 f32)
SL = 4224            # strip length: self block + 32 forward blocks
CW = 4096            # colsum window end (blocks 1..31): [base+128, base+CW)
PT = 1024            # PSUM strip tile columns (2 banks)
UCOLS = 5120         # rotated cols touched: [0, 128*(S-1) + SL) = 5120
NCH = 10             # colsum chunks of 512 covering rotated cols [0, 5120)
LARGE = 7296.0       # diagonal nuke: d2 -> 7296, w -> 85.4:
                     #   ACT path exp(-85.4) ~ 8e-38 (bf16 ~ 0)
                     #   DVE path code = B16 - A16*85.4 ~ +474 (tiny value)
N_DVE = 5            # strips whose exp runs on DVE (Schraudolph)

LN2 = float(np.log(2.0))
A16 = 128.0 / LN2    # bf16 exponent-code slope
B16 = 16248.0        # bf16 exponent-code offset (tuned: ~zero mean bias)

PROFILE = False
LAST_RESULT = None

_cache = {}


def _build_program():
    import concourse.tile as tile
    from bass_rust import add_dep_helper
    from concourse import bacc, mybir

    f32 = mybir.dt.float32
    f16 = mybir.dt.float16
    bf16 = mybir.dt.bfloat16
    i16 = mybir.dt.int16
    fp8 = mybir.dt.float8e4
    AF = mybir.ActivationFunctionType
    OP = mybir.AluOpType
    PM = mybir.MatmulPerfMode

    nc = bacc.Bacc()

    h_u2 = nc.declare_dram_parameter("u2", [65, 2, UCOLS], fp8, isOutput=False)
    h_u2w = nc.declare_dram_parameter("u2w", [65, 2, S * 128], fp8, isOutput=False)
    h_dfix = nc.declare_dram_parameter("dfix", [128, 128], bf16, isOutput=False)
    h_ident = nc.declare_dram_parameter("ident", [128, 128], bf16, isOutput=False)
    h_oneh = nc.declare_dram_parameter(
        "oneh", [128, NCH * NCH], bf16, isOutput=False
    )
    h_hsqp = nc.declare_dram_parameter("hsqp", [128, S], f32, isOutput=False)
    h_hsqpa = nc.declare_dram_parameter("hsqpa", [128, S], f32, isOutput=False)
    h_s0 = nc.declare_dram_parameter("s0", [128, S], f32, isOutput=True)
    h_cs = nc.declare_dram_parameter("cs", [NCH, CT], f32, isOutput=True)

    dve_strip = [s < N_DVE for s in range(S)]

    # strip for subtile s covers rotated cols [s*128, s*128 + SL)
    with tile.TileContext(nc) as tc, ExitStack() as ctx:
        const = ctx.enter_context(tc.tile_pool(name="const", bufs=1))
        wpool = ctx.enter_context(tc.tile_pool(name="wbuf", bufs=S))
        dpool = ctx.enter_context(tc.tile_pool(name="dump", bufs=2))
        cpool = ctx.enter_context(tc.tile_pool(name="codes", bufs=2))
        pspool = ctx.enter_context(tc.tile_pool(name="ps", bufs=3, space="PSUM"))
        pstail = ctx.enter_context(tc.tile_pool(name="pst", bufs=1, space="PSUM"))
        cspool = ctx.enter_context(tc.tile_pool(name="cps", bufs=1, space="PSUM"))
        misc = ctx.enter_context(tc.tile_pool(name="misc", bufs=1))

        # small operands first (cheap, needed early)
        t_dfix = const.tile([128, 128], bf16)
        nc.sync.dma_start(out=t_dfix[:], in_=h_dfix[:])
        t_ident = const.tile([128, 128], bf16)
        nc.sync.dma_start(out=t_ident[:], in_=h_ident[:])
        t_oneh = const.tile([128, NCH * NCH], bf16)
        nc.sync.dma_start(out=t_oneh[:], in_=h_oneh[:])
        t_hsqp = const.tile([128, S], f32)
        nc.sync.dma_start(out=t_hsqp[:], in_=h_hsqp[:])
        t_hsqpa = const.tile([128, S], f32)
        nc.sync.dma_start(out=t_hsqpa[:], in_=h_hsqpa[:])
        t_u2w = const.tile([65, 2, S * 128], fp8)
        nc.sync.dma_start(out=t_u2w[:], in_=h_u2w[:])

        # big operand: fine-grained chunks, first strip's columns first
        t_u2 = const.tile([65, 2, UCOLS], fp8)
        edges = [0, 256, 512, 768, 1024, 1536, 2048, 2560, 3072, 3584,
                 4096, 4608, 5120]
        for a, b in zip(edges[:-1], edges[1:]):
            nc.sync.dma_start(out=t_u2[:, :, a:b], in_=h_u2[:, :, a:b])

        t_zero10 = const.tile([128, NCH], bf16)
        nc.vector.memset(t_zero10[:], 0.0)
        t_z512 = const.tile([128, CT], bf16)
        nc.vector.memset(t_z512[:], 0.0)
        t_b16 = const.tile([128, SL], f16)
        nc.vector.memset(t_b16[:], B16)

        s0_t = const.tile([128, S], f32)
        junk = const.tile([128, 1], f32)
        scratch = const.tile([128, SL], bf16)

        # single resident colsum accumulator [NCH, 512]
        cs_acc = cspool.tile([NCH, CT], f32, tag="cs", name="cs_acc")

        # zero the colsum accumulator (matmul with zero weights) and keep
        # the PE busy a few us so the clock gate opens before real work
        for rep in range(8):
            nc.tensor.matmul(
                cs_acc[:], t_zero10[:], t_z512[:],
                start=(rep == 0), stop=False, skip_group_check=True,
            )

        # ---- Sqrt phase: PE DR-65 fp8 matmuls -> PSUM, ACT sqrts ----
        ws = []
        last_sqrt = None
        for s in range(S):
            base = s * 128  # strip start in rotated cols
            w = wpool.tile([128, SL], bf16, tag="w")
            ws.append(w)
            lw = t_u2w[:, :, base:base + 128]
            if dve_strip[s]:
                # Schraudolph path: w holds s-codes' source A16*d
                scale = -(A16 * A16)
                bias = t_hsqpa[:, s:s + 1]
            else:
                scale = -1.0
                bias = t_hsqp[:, s:s + 1]
            for t in range(4):  # four 1024-col PSUM tiles
                c0 = t * PT
                ps = pspool.tile([128, PT], f32, tag="ps")
                for q0 in range(c0, c0 + PT, CT):
                    nc.tensor.matmul(
                        ps[:, q0 - c0:q0 - c0 + CT],
                        lw,
                        t_u2[:, :, base + q0:base + q0 + CT],
                        start=True,
                        stop=not (t == 0 and q0 == 0),
                        perf_mode=PM.DoubleRow,
                    )
                    if t == 0 and q0 == 0:
                        # self block: nuke the diagonal (cols [0,128))
                        nc.tensor.matmul(
                            ps[:, 0:128],
                            t_ident[:],
                            t_dfix[:],
                            start=False,
                            stop=True,
                            skip_group_check=True,
                        )
                # w = sqrt(hsq_i - Q) (= d_ij, or A16*d_ij on DVE strips)
                nc.scalar.activation(
                    out=w[:, c0:c0 + PT],
                    in_=ps[:],
                    func=AF.Sqrt,
                    scale=scale,
                    bias=bias,
                )
            # antipodal 128-col tail
            pst = pstail.tile([128, 128], f32, tag="pst")
            nc.tensor.matmul(
                pst[:],
                lw,
                t_u2[:, :, base + SL - 128:base + SL],
                start=True,
                stop=True,
                perf_mode=PM.DoubleRow,
            )
            last_sqrt = nc.scalar.activation(
                out=w[:, SL - 128:SL],
                in_=pst[:],
                func=AF.Sqrt,
                scale=scale,
                bias=bias,
            )

        # ---- Exp phase + column sums ----
        # DVE strips run the Schraudolph pair as soon as their w exists;
        # ACT strips wait for the sqrt->exp table switch.
        def colsums(s, etile):
            base = s * 128
            lo = base + 128
            hi = base + CW
            j = lo // CT
            while j * CT < hi:
                a = max(lo, j * CT)
                b = min(hi, (j + 1) * CT)
                nc.tensor.matmul(
                    cs_acc[:, a - j * CT:b - j * CT],
                    t_oneh[:, NCH * j:NCH * (j + 1)],
                    etile[:, a - base:b - base],
                    start=False,
                    stop=False,
                    skip_group_check=True,
                )
                j += 1

        for s in range(S):
            if not dve_strip[s]:
                continue
            w = ws[s]
            codes = cpool.tile([128, SL], i16, tag="codes")
            # codes = int16((w * -1) + B16) = bf16 bits of ~exp(-d)
            nc.vector.scalar_tensor_tensor(
                out=codes[:], in0=w[:], scalar=-1.0,
                in1=t_b16[:], op0=OP.mult, op1=OP.add,
                accum_out=junk[:],
            )
            # row sums of the decoded bf16 values
            nc.vector.tensor_scalar(
                out=scratch[:], in0=codes[:].bitcast(bf16),
                scalar1=1.0, scalar2=0.0,
                op0=OP.mult, op1=OP.add,
                accum_out=s0_t[:, s:s + 1],
            )
            colsums(s, codes[:].bitcast(bf16))

        for s in range(S):
            if dve_strip[s]:
                continue
            w = ws[s]
            dump = dpool.tile([128, SL], bf16, tag="dump")
            e = nc.scalar.activation(
                out=dump[:],
                in_=w[:],
                func=AF.Exp,
                scale=-1.0,
                accum_out=s0_t[:, s:s + 1],
            )
            if last_sqrt is not None:
                add_dep_helper(
                    e.ins, last_sqrt.ins, sync=False,
                    reason="ACT table phase: exp after all sqrts",
                )
            colsums(s, dump[:])

        # drain colsum accumulator: PSUM -> SBUF -> DRAM
        sb = misc.tile([NCH, CT], f32, tag="csdrain")
        nc.vector.tensor_copy(sb[:], cs_acc[:])
        nc.sync.dma_start(out=h_cs[:], in_=sb[:])
        nc.sync.dma_start(out=h_s0[:], in_=s0_t[:])

    nc.finalize()
    return nc


def get_program():
    if "nc" not in _cache:
        _cache["nc"] = _build_program()
    return _cache["nc"]


def make_in_maps(x, y):
    """Host-side prep: build the per-core (column-rotated) operand arrays."""
    from concourse import mybir

    FP8 = np.dtype(mybir.dt.np(mybir.dt.float8e4))

    x = np.asarray(x, dtype=np.float32)
    y = np.asarray(y, dtype=np.float32)
    z = np.concatenate([x, y], axis=0)  # [N, D]

    u8 = (np.float32(np.sqrt(2.0)) * z).astype(FP8)        # [N, D] fp8
    uf = u8.astype(np.float32)
    hsq = np.float32(0.5) * (uf * uf).sum(axis=1, dtype=np.float32)
    hsq_hi = hsq.astype(FP8)
    hsq_lo = (hsq - hsq_hi.astype(np.float32)).astype(FP8)

    ut = np.ascontiguousarray(uf.T)  # [D, N] f32 of the fp8 values

    dfix = np.zeros((128, 128), dtype=BF16)
    idx = np.arange(128)
    dfix[idx, idx] = BF16(-LARGE)
    ident = np.eye(128, dtype=BF16)
    oneh = np.zeros((128, NCH * NCH), dtype=BF16)
    for j in range(NCH):
        oneh[:, NCH * j + j] = BF16(1.0)

    in_maps = []
    for c in range(NCORES):
        r0 = c * RPC
        rows = np.arange(r0, r0 + RPC)

        def rotc(a):  # rotate columns of [*, N] by -r0, crop to UCOLS
            return np.roll(a, -r0, axis=-1)[..., :UCOLS]

        utr = rotc(ut)                       # [128, UCOLS] f32
        hhr = rotc(hsq_hi[None, :])[0]       # [UCOLS] fp8
        hlr = rotc(hsq_lo[None, :])[0]       # [UCOLS] fp8

        u2 = np.zeros((65, 2, UCOLS), dtype=FP8)
        u2[0:64, 0, :] = utr[0:64].astype(FP8)
        u2[0:64, 1, :] = utr[64:128].astype(FP8)
        u2[64, 0, :] = -hhr
        u2[64, 1, :] = -hlr

        u2w = np.zeros((65, 2, S * 128), dtype=FP8)
        u2w[0:64, 0, :] = utr[0:64, :S * 128].astype(FP8)
        u2w[0:64, 1, :] = utr[64:128, :S * 128].astype(FP8)
        u2w[64, 0, :] = np.float32(1.0).astype(FP8)
        u2w[64, 1, :] = np.float32(1.0).astype(FP8)

        def pcol(vec, sel):  # [RPC] values -> [128, S] per-partition layout
            return np.ascontiguousarray(vec[sel].reshape(S, 128).T)

        hp = pcol(hsq, rows)
        in_maps.append(
            {
                "u2": u2,
                "u2w": u2w,
                "dfix": dfix,
                "ident": ident,
                "oneh": oneh,
                "hsqp": hp,
                "hsqpa": np.ascontiguousarray(hp * np.float32(A16 * A16)),
            }
        )
    return in_maps


def finish_on_host(results, x, y):
    """Gather per-core row sums + column sums; final loss with host dp."""
    S0 = np.zeros(N, dtype=np.float64)
    for c in range(NCORES):
        r0 = c * RPC
        s0 = np.asarray(results[c]["s0"], dtype=np.float64)  # [128, S]
        cs = np.asarray(results[c]["cs"], dtype=np.float64)  # [NCH, CT]
        S0[r0:r0 + RPC] += s0.T.reshape(-1)
        # accumulated column sums: rotated col r in [128, 4992) holds the
        # core's total colsum for global row (r0 + r) mod N
        csf = cs.reshape(-1)
        rot = np.arange(128, (S - 1) * 128 + CW)
        gidx = (r0 + rot) % N
        S0[gidx] += csf[rot]

    z = np.concatenate([np.asarray(x, np.float64), np.asarray(y, np.float64)])
    dp = np.sqrt(((z[:B] - z[B:]) ** 2).sum(axis=1))
    DP = np.concatenate([dp, dp])

    tiny = float(np.finfo(np.float32).tiny)
    num = np.exp(-DP)
    loss = -np.log(num / S0 + tiny)
    return np.asarray(loss.mean(), dtype=np.float32)


def kernel(x, y):
    global LAST_RESULT
    from concourse.bass_utils import run_bass_kernel_spmd

    nc = get_program()
    in_maps = make_in_maps(x, y)
    res = run_bass_kernel_spmd(
        nc, in_maps, list(range(NCORES)), trace=PROFILE
    )
    LAST_RESULT = res
    return finish_on_host(res.results, x, y)


# revision 21
# speedup vs baseline: 1.5257x; 1.0211x over previous
"""SNN (soft-nearest-neighbor) contrastive loss on 8 Trainium2 NeuronCores.

Math
----
z = concat(x, y) in R^{8192x128};  d_ij = ||z_i - z_j||.
The row max subtracted in the reference cancels mathematically, so
    S0_i  = sum_{j != i} exp(-d_ij)          (device + host gather)
    dp_i  = d_{i, pair(i)}                   (host, O(N*D))
    loss  = mean_i( -log( exp(-dp_i)/S0_i + tiny ) )   (host, trivial)

Symmetry halving
----------------
d_ij is symmetric; each 128-row block R computes exp tiles for column
blocks R..R+32 only (self + 32 forward, cyclically).  Strip = 4224 cols.
Row sums cover the WHOLE strip (the antipodal block, offset 32, is
computed by both partners for their own rows).  Column sums (PE one-hot
matmul into a single [10,512] PSUM accumulator) cover offsets 1..31 and
are scattered on the host into the mirrored rows.

Device pipeline (one SPMD program, 8 cores, rows sharded 1024/core)
------------------------------------------------------------------
PE: fp8 DoubleRow matmuls with 65-row k-tiles compute
      Q = u.u^T - hsq_j      (u split into 2x64 dims; the 65th row of
k-tile 0/1 carries ones x -hsq_hi / ones x -hsq_lo, giving the hsq_j
fold at fp16-ish precision for free), plus an FD-128 bf16 identity
matmul adding -LARGE on the self diagonal.
ACT: w = Sqrt(-Q + hsq_i) straight from PSUM via the per-partition
bias AP (bf16 out).  Exp is split: N_ACT strips run on ACT
(exp + fused accum_out row sums); N_DVE strips run on DVE via the
Schraudolph bit trick -- the sqrt for those strips is scaled by A16^2
so codes = int16(B16 - A16*w) come from one scalar_tensor_tensor, and
a second tensor_scalar pass over the bitcast-bf16 codes yields row
sums via accum_out.  PE accumulates column sums of every strip.
Each core gets column-ROTATED operands so every tile index is a
compile-time constant: one identical program for all 8 cores.
"""

import os
import sys
from contextlib import ExitStack

import numpy as np

_TRN_REPO = os.environ.get("TRN_RL_REPO", "/opt/trn_rl_repo")
if _TRN_REPO not in sys.path:
    sys.path.insert(0, _TRN_REPO)

import ml_dtypes

BF16 = ml_dtypes.bfloat16

B = 4096
D = 128
N = 2 * B            # 8192 rows of z
NCORES = 8
RPC = N // NCORES    # 1024 rows per core
S = RPC // 128       # 8 row-subtiles per core
CT = 512             # matmul moving tile (one PSUM bank = 512 f32)
SL = 4224            # strip length: self block + 32 forward blocks
CW = 4096            # colsum window end (blocks 1..31): [base+128, base+CW)
PT = 1024            # PSUM strip tile columns (2 banks)
UCOLS = 5120         # rotated cols touched: [0, 128*(S-1) + SL) = 5120
NCH = 10             # colsum chunks of 512 covering rotated cols [0, 5120)
LARGE = 7296.0       # diagonal nuke: d2 -> 7296, w -> 85.4:
                     #   ACT path exp(-85.4) ~ 8e-38 (bf16 ~ 0)
                     #   DVE path code = B16 - A16*85.4 ~ +474 (tiny value)
N_DVE = 6            # strips whose exp runs on DVE (Schraudolph)

LN2 = float(np.log(2.0))
A16 = 128.0 / LN2    # bf16 exponent-code slope
B16 = 16256.0        # bf16 exponent-code offset (exact in bf16)
# with B16=16256 the Schraudolph decode averages exp(-w)*SCALE_COMP;
# ACT-strip exps are biased by ln(SCALE_COMP) to match, and the host
# divides all device sums by SCALE_COMP.
SCALE_COMP = 1.0406027025852233  # mean of (1+f)/2^f over f~U[0,1)

PROFILE = False
LAST_RESULT = None

_cache = {}


def _build_program():
    import concourse.tile as tile
    from bass_rust import add_dep_helper
    from concourse import bacc, mybir

    f32 = mybir.dt.float32
    f16 = mybir.dt.float16
    bf16 = mybir.dt.bfloat16
    i16 = mybir.dt.int16
    fp8 = mybir.dt.float8e4
    AF = mybir.ActivationFunctionType
    OP = mybir.AluOpType
    PM = mybir.MatmulPerfMode

    nc = bacc.Bacc()

    h_u2 = nc.declare_dram_parameter("u2", [65, 2, UCOLS], fp8, isOutput=False)
    h_u2w = nc.declare_dram_parameter("u2w", [65, 2, S * 128], fp8, isOutput=False)
    h_dfix = nc.declare_dram_parameter("dfix", [128, 128], bf16, isOutput=False)
    h_ident = nc.declare_dram_parameter("ident", [128, 128], bf16, isOutput=False)
    h_oneh = nc.declare_dram_parameter(
        "oneh", [128, NCH * NCH], bf16, isOutput=False
    )
    h_hsqp = nc.declare_dram_parameter("hsqp", [128, S], f32, isOutput=False)
    h_hsqpa = nc.declare_dram_parameter("hsqpa", [128, S], f32, isOutput=False)
    h_s0 = nc.declare_dram_parameter("s0", [128, S], f32, isOutput=True)
    h_cs = nc.declare_dram_parameter("cs", [NCH, CT], f32, isOutput=True)

    dve_strip = [s < N_DVE for s in range(S)]

    # strip for subtile s covers rotated cols [s*128, s*128 + SL)
    with tile.TileContext(nc) as tc, ExitStack() as ctx:
        const = ctx.enter_context(tc.tile_pool(name="const", bufs=1))
        wpool = ctx.enter_context(tc.tile_pool(name="wbuf", bufs=S))
        dpool = ctx.enter_context(tc.tile_pool(name="dump", bufs=2))
        cpool = ctx.enter_context(tc.tile_pool(name="codes", bufs=2))
        pspool = ctx.enter_context(tc.tile_pool(name="ps", bufs=3, space="PSUM"))
        pstail = ctx.enter_context(tc.tile_pool(name="pst", bufs=1, space="PSUM"))
        cspool = ctx.enter_context(tc.tile_pool(name="cps", bufs=1, space="PSUM"))
        misc = ctx.enter_context(tc.tile_pool(name="misc", bufs=1))

        # small operands first (cheap, needed early)
        t_dfix = const.tile([128, 128], bf16)
        nc.sync.dma_start(out=t_dfix[:], in_=h_dfix[:])
        t_ident = const.tile([128, 128], bf16)
        nc.sync.dma_start(out=t_ident[:], in_=h_ident[:])
        t_oneh = const.tile([128, NCH * NCH], bf16)
        nc.sync.dma_start(out=t_oneh[:], in_=h_oneh[:])
        t_hsqp = const.tile([128, S], f32)
        nc.sync.dma_start(out=t_hsqp[:], in_=h_hsqp[:])
        t_hsqpa = const.tile([128, S], f32)
        nc.sync.dma_start(out=t_hsqpa[:], in_=h_hsqpa[:])
        t_u2w = const.tile([65, 2, S * 128], fp8)
        nc.sync.dma_start(out=t_u2w[:], in_=h_u2w[:])

        # big operand: fine-grained chunks, first strip's columns first
        t_u2 = const.tile([65, 2, UCOLS], fp8)
        edges = [0, 256, 512, 768, 1024, 1536, 2048, 2560, 3072, 3584,
                 4096, 4608, 5120]
        for a, b in zip(edges[:-1], edges[1:]):
            nc.sync.dma_start(out=t_u2[:, :, a:b], in_=h_u2[:, :, a:b])

        t_zero10 = const.tile([128, NCH], bf16)
        nc.vector.memset(t_zero10[:], 0.0)
        t_z512 = const.tile([128, CT], bf16)
        nc.vector.memset(t_z512[:], 0.0)
        t_b16 = const.tile([128, SL], bf16)
        nc.vector.memset(t_b16[:], B16)
        t_ebias = const.tile([128, 1], f32)
        nc.vector.memset(t_ebias[:], float(np.log(SCALE_COMP)))

        s0_t = const.tile([128, S], f32)
        junk = const.tile([128, 1], f32)
        scratch = const.tile([128, SL], bf16)

        # single resident colsum accumulator [NCH, 512]
        cs_acc = cspool.tile([NCH, CT], f32, tag="cs", name="cs_acc")

        # zero the colsum accumulator (matmul with zero weights) and keep
        # the PE busy a few us so the clock gate opens before real work
        for rep in range(8):
            nc.tensor.matmul(
                cs_acc[:], t_zero10[:], t_z512[:],
                start=(rep == 0), stop=False, skip_group_check=True,
            )

        # ---- Sqrt phase: PE DR-65 fp8 matmuls -> PSUM, ACT sqrts ----
        ws = []
        last_sqrt = None
        for s in range(S):
            base = s * 128  # strip start in rotated cols
            w = wpool.tile([128, SL], bf16, tag="w")
            ws.append(w)
            lw = t_u2w[:, :, base:base + 128]
            if dve_strip[s]:
                # Schraudolph path: w holds s-codes' source A16*d
                scale = -(A16 * A16)
                bias = t_hsqpa[:, s:s + 1]
            else:
                scale = -1.0
                bias = t_hsqp[:, s:s + 1]
            for t in range(4):  # four 1024-col PSUM tiles
                c0 = t * PT
                ps = pspool.tile([128, PT], f32, tag="ps")
                for q0 in range(c0, c0 + PT, CT):
                    nc.tensor.matmul(
                        ps[:, q0 - c0:q0 - c0 + CT],
                        lw,
                        t_u2[:, :, base + q0:base + q0 + CT],
                        start=True,
                        stop=not (t == 0 and q0 == 0),
                        perf_mode=PM.DoubleRow,
                    )
                    if t == 0 and q0 == 0:
                        # self block: nuke the diagonal (cols [0,128))
                        nc.tensor.matmul(
                            ps[:, 0:128],
                            t_ident[:],
                            t_dfix[:],
                            start=False,
                            stop=True,
                            skip_group_check=True,
                        )
                # w = sqrt(hsq_i - Q) (= d_ij, or A16*d_ij on DVE strips)
                nc.scalar.activation(
                    out=w[:, c0:c0 + PT],
                    in_=ps[:],
                    func=AF.Sqrt,
                    scale=scale,
                    bias=bias,
                )
            # antipodal 128-col tail
            pst = pstail.tile([128, 128], f32, tag="pst")
            nc.tensor.matmul(
                pst[:],
                lw,
                t_u2[:, :, base + SL - 128:base + SL],
                start=True,
                stop=True,
                perf_mode=PM.DoubleRow,
            )
            last_sqrt = nc.scalar.activation(
                out=w[:, SL - 128:SL],
                in_=pst[:],
                func=AF.Sqrt,
                scale=scale,
                bias=bias,
            )

        # ---- Exp phase + column sums ----
        # DVE strips run the Schraudolph pair as soon as their w exists;
        # ACT strips wait for the sqrt->exp table switch.
        def colsums(s, etile):
            base = s * 128
            lo = base + 128
            hi = base + CW
            j = lo // CT
            while j * CT < hi:
                a = max(lo, j * CT)
                b = min(hi, (j + 1) * CT)
                nc.tensor.matmul(
                    cs_acc[:, a - j * CT:b - j * CT],
                    t_oneh[:, NCH * j:NCH * (j + 1)],
                    etile[:, a - base:b - base],
                    start=False,
                    stop=False,
                    skip_group_check=True,
                )
                j += 1

        for s in range(S):
            if not dve_strip[s]:
                continue
            w = ws[s]
            codes = cpool.tile([128, SL], i16, tag="codes")
            # codes = int16((w * -1) + B16) = bf16 bits of ~exp(-d)
            nc.vector.scalar_tensor_tensor(
                out=codes[:], in0=w[:], scalar=-1.0,
                in1=t_b16[:], op0=OP.mult, op1=OP.add,
                accum_out=junk[:],
            )
            # row sums of the decoded bf16 values
            nc.vector.tensor_scalar(
                out=scratch[:], in0=codes[:].bitcast(bf16),
                scalar1=1.0, scalar2=0.0,
                op0=OP.mult, op1=OP.add,
                accum_out=s0_t[:, s:s + 1],
            )
            colsums(s, codes[:].bitcast(bf16))

        for s in range(S):
            if dve_strip[s]:
                continue
            w = ws[s]
            dump = dpool.tile([128, SL], bf16, tag="dump")
            e = nc.scalar.activation(
                out=dump[:],
                in_=w[:],
                func=AF.Exp,
                scale=-1.0,
                bias=t_ebias[:],
                accum_out=s0_t[:, s:s + 1],
            )
            if last_sqrt is not None:
                add_dep_helper(
                    e.ins, last_sqrt.ins, sync=False,
                    reason="ACT table phase: exp after all sqrts",
                )
            colsums(s, dump[:])

        # drain colsum accumulator: PSUM -> SBUF -> DRAM
        sb = misc.tile([NCH, CT], f32, tag="csdrain")
        nc.vector.tensor_copy(sb[:], cs_acc[:])
        nc.sync.dma_start(out=h_cs[:], in_=sb[:])
        nc.sync.dma_start(out=h_s0[:], in_=s0_t[:])

    nc.finalize()
    return nc


def get_program():
    if "nc" not in _cache:
        _cache["nc"] = _build_program()
    return _cache["nc"]


def make_in_maps(x, y):
    """Host-side prep: build the per-core (column-rotated) operand arrays."""
    from concourse import mybir

    FP8 = np.dtype(mybir.dt.np(mybir.dt.float8e4))

    x = np.asarray(x, dtype=np.float32)
    y = np.asarray(y, dtype=np.float32)
    z = np.concatenate([x, y], axis=0)  # [N, D]

    u8 = (np.float32(np.sqrt(2.0)) * z).astype(FP8)        # [N, D] fp8
    uf = u8.astype(np.float32)
    hsq = np.float32(0.5) * (uf * uf).sum(axis=1, dtype=np.float32)
    hsq_hi = hsq.astype(FP8)
    hsq_lo = (hsq - hsq_hi.astype(np.float32)).astype(FP8)

    ut = np.ascontiguousarray(uf.T)  # [D, N] f32 of the fp8 values

    dfix = np.zeros((128, 128), dtype=BF16)
    idx = np.arange(128)
    dfix[idx, idx] = BF16(-LARGE)
    ident = np.eye(128, dtype=BF16)
    oneh = np.zeros((128, NCH * NCH), dtype=BF16)
    for j in range(NCH):
        oneh[:, NCH * j + j] = BF16(1.0)

    in_maps = []
    for c in range(NCORES):
        r0 = c * RPC
        rows = np.arange(r0, r0 + RPC)

        def rotc(a):  # rotate columns of [*, N] by -r0, crop to UCOLS
            return np.roll(a, -r0, axis=-1)[..., :UCOLS]

        utr = rotc(ut)                       # [128, UCOLS] f32
        hhr = rotc(hsq_hi[None, :])[0]       # [UCOLS] fp8
        hlr = rotc(hsq_lo[None, :])[0]       # [UCOLS] fp8

        u2 = np.zeros((65, 2, UCOLS), dtype=FP8)
        u2[0:64, 0, :] = utr[0:64].astype(FP8)
        u2[0:64, 1, :] = utr[64:128].astype(FP8)
        u2[64, 0, :] = -hhr
        u2[64, 1, :] = -hlr

        u2w = np.zeros((65, 2, S * 128), dtype=FP8)
        u2w[0:64, 0, :] = utr[0:64, :S * 128].astype(FP8)
        u2w[0:64, 1, :] = utr[64:128, :S * 128].astype(FP8)
        u2w[64, 0, :] = np.float32(1.0).astype(FP8)
        u2w[64, 1, :] = np.float32(1.0).astype(FP8)

        def pcol(vec, sel):  # [RPC] values -> [128, S] per-partition layout
            return np.ascontiguousarray(vec[sel].reshape(S, 128).T)

        hp = pcol(hsq, rows)
        in_maps.append(
            {
                "u2": u2,
                "u2w": u2w,
                "dfix": dfix,
                "ident": ident,
                "oneh": oneh,
                "hsqp": hp,
                "hsqpa": np.ascontiguousarray(hp * np.float32(A16 * A16)),
            }
        )
    return in_maps


def finish_on_host(results, x, y):
    """Gather per-core row sums + column sums; final loss with host dp."""
    S0 = np.zeros(N, dtype=np.float64)
    for c in range(NCORES):
        r0 = c * RPC
        s0 = np.asarray(results[c]["s0"], dtype=np.float64)  # [128, S]
        cs = np.asarray(results[c]["cs"], dtype=np.float64)  # [NCH, CT]
        S0[r0:r0 + RPC] += s0.T.reshape(-1)
        # accumulated column sums: rotated col r in [128, 4992) holds the
        # core's total colsum for global row (r0 + r) mod N
        csf = cs.reshape(-1)
        rot = np.arange(128, (S - 1) * 128 + CW)
        gidx = (r0 + rot) % N
        S0[gidx] += csf[rot]

    z = np.concatenate([np.asarray(x, np.float64), np.asarray(y, np.float64)])
    dp = np.sqrt(((z[:B] - z[B:]) ** 2).sum(axis=1))
    DP = np.concatenate([dp, dp])

    S0 /= SCALE_COMP
    tiny = float(np.finfo(np.float32).tiny)
    num = np.exp(-DP)
    loss = -np.log(num / S0 + tiny)
    return np.asarray(loss.mean(), dtype=np.float32)


def kernel(x, y):
    global LAST_RESULT
    from concourse.bass_utils import run_bass_kernel_spmd

    nc = get_program()
    in_maps = make_in_maps(x, y)
    res = run_bass_kernel_spmd(
        nc, in_maps, list(range(NCORES)), trace=PROFILE
    )
    LAST_RESULT = res
    return finish_on_host(res.results, x, y)
